# revision 7
# baseline (speedup 1.0000x reference)
"""Multi-head attention (B=2, S=2048, E=768, H=12, D=64) on 8 NeuronCores.

Sharding: core c -> batch b = c//4, head group hg = c%4 (3 heads each).
Each core computes qkv projection for its 3 heads, attention, and a partial
output projection (rows of w_proj for its heads). Host sums the 4 partials
per batch and adds the bias terms (tensor-parallel unshard).

Device dataflow (everything transposed so no on-chip transposes are needed):
  xT [768, 2048]  (host-pretransposed, fp16)
  qkT[t] = (w_qk_tile_t)^T @ xT   -> [128, 2048] tiles t=0..2, cols ordered
           [q0|q1], [k0|k1], [q2|k2]; k2 moved to partitions 0:64 by an
           SBUF->SBUF DMA so every head's q,k share a base partition.
  v    = xT_tile^T @ w_v          -> [S, 192] -> per-head v' [128, 65] blocks
           (col 64 = ones, so AV also produces the softmax row sums)
  pT   = exp(0.125 * kT_tile^T @ qT)   (ScalarE, PSUM->SBUF fp16)
  avT  = v'^T @ pT                -> [65, 512] PSUM per (head, Sq-chunk);
           row 64 = softmax denominator l
  outT = avT[0:64] * (1/l)        (DVE; 1/l broadcast across partitions
           via an SBUF->SBUF DMA with a 0-stride partition AP)
  yT  += w_proj_part^T @ outT     -> [768, 2048] fp32 partial, DMA'd out.
"""

import threading

import numpy as np
import ml_dtypes

import concourse.bass as bass
import concourse.tile as tile
from concourse import bacc, mybir
from concourse.bass import ts, ds
from concourse.bass_utils import run_bass_kernel_spmd

F32 = mybir.dt.float32
F16 = mybir.dt.float16

EMBED = 768
NH = 12
D = 64
B = 2
S = 2048
HPC = 3          # heads per core
NCORES = 8
P = 128
KC = EMBED // P  # 6 contraction chunks
NQ = S // 512    # 4 Sq chunks of 512


def _build_kernel(nc):
    xT = nc.dram_tensor("xT", [EMBED, S], F16, kind="ExternalInput").ap()
    wqk = nc.dram_tensor("w_qk", [EMBED, 2 * HPC * D], F16, kind="ExternalInput").ap()
    bqk = nc.dram_tensor("b_qk", [2 * HPC * D], F32, kind="ExternalInput").ap()
    wv = nc.dram_tensor("w_v", [EMBED, HPC * D], F16, kind="ExternalInput").ap()
    wp = nc.dram_tensor("w_p", [HPC * D, EMBED], F16, kind="ExternalInput").ap()
    yT = nc.dram_tensor("yT", [EMBED, S], F32, kind="ExternalOutput").ap()

    with tile.TileContext(nc) as tc:
        with (
            tc.tile_pool(name="wpool", bufs=1) as wpool,
            tc.tile_pool(name="xpool", bufs=1) as xpool,
            tc.tile_pool(name="qkpool", bufs=1) as qkpool,
            tc.tile_pool(name="vpool", bufs=1) as vpool,
            tc.tile_pool(name="ptpool", bufs=20) as ptpool,
            tc.tile_pool(name="opool", bufs=1) as opool,
            tc.tile_pool(name="rlpool", bufs=2) as rlpool,
            tc.tile_pool(name="scrpool", bufs=2, space="DRAM") as scrpool,
            tc.tile_pool(name="psum", bufs=2, space="PSUM") as psum,
        ):
            # ---- loads ----
            wqk_t = []
            wv_t = []
            xT_t = []
            bq_t = []
            for k in range(KC):
                wqk_k = wpool.tile([P, 2 * HPC * D], F16, name=f"wqk{k}")
                nc.sync.dma_start(out=wqk_k, in_=wqk[ts(k, P), :])
                wqk_t.append(wqk_k)
            for k in range(KC):
                wv_k = wpool.tile([P, HPC * D], F16, name=f"wv{k}")
                nc.sync.dma_start(out=wv_k, in_=wv[ts(k, P), :])
                wv_t.append(wv_k)
            wpA = wpool.tile([P, EMBED], F16)
            nc.sync.dma_start(out=wpA, in_=wp[0:P, :])
            wpB = wpool.tile([D, EMBED], F16)
            nc.sync.dma_start(out=wpB, in_=wp[P : P + D, :])
            for t in range(HPC):
                bq_k = wpool.tile([P, 1], F32, name=f"bq{t}")
                nc.sync.dma_start(
                    out=bq_k, in_=bqk[ts(t, P)].rearrange("(p o) -> p o", o=1)
                )
                bq_t.append(bq_k)
            for k in range(KC):
                xT_k = xpool.tile([P, S], F16, name=f"xT{k}")
                nc.sync.dma_start(out=xT_k, in_=xT[ts(k, P), :])
                xT_t.append(xT_k)

            # ---- qk projection: qkT[t] [128, S], cols [q0|q1],[k0|k1],[q2|k2]
            qkT = []
            for t in range(HPC):
                qkT_i = qkpool.tile([P, S], F16, name=f"qkT{t}")
                for nq in range(NQ):
                    ps = psum.tile([P, 512], F32, tag="sc", name="ps_qk")
                    for k in range(KC):
                        nc.tensor.matmul(
                            ps,
                            lhsT=wqk_t[k][:, ts(t, P)],
                            rhs=xT_t[k][:, ts(nq, 512)],
                            start=(k == 0),
                            stop=(k == KC - 1),
                        )
                    nc.vector.tensor_scalar_add(
                        out=qkT_i[:, ts(nq, 512)], in0=ps, scalar1=bq_t[t]
                    )
                qkT.append(qkT_i)

            # k2 lives at partitions 64:128 of qkT[2]; move to base 0.
            k2f = qkpool.tile([D, S], F16)
            nc.sync.dma_start(out=k2f, in_=qkT[2][D:P, :])

            # ---- v projection -> v' per head: [128, 16*65] fp16 (col 64=1.0)
            vp = []
            for h in range(HPC):
                vp_h = vpool.tile([P, 16 * (D + 1)], F16, name=f"vp{h}")
                nc.vector.memset(
                    vp_h.rearrange("p (s c) -> p s c", c=D + 1)[:, :, D], 1.0
                )
                vp.append(vp_h)
            for st in range(S // P):
                vps = psum.tile([P, HPC * D], F32, tag="sc", name="ps_v")
                for k in range(KC):
                    nc.tensor.matmul(
                        vps,
                        lhsT=xT_t[k][:, ts(st, P)],
                        rhs=wv_t[k],
                        start=(k == 0),
                        stop=(k == KC - 1),
                    )
                for h in range(HPC):
                    nc.vector.tensor_copy(
                        out=vp[h][:, ds(st * (D + 1), D)], in_=vps[:, ts(h, D)]
                    )

            # per-head q/k sources: (tile, partition offset)
            qsrc = [(qkT[0], 0), (qkT[0], D), (qkT[2], 0)]
            ksrc = [(qkT[1], 0), (qkT[1], D), (k2f, 0)]

            # out_headsT destinations
            st01 = opool.tile([P, S], F16)   # heads 0 (rows 0:64) and 1 (64:128)
            outT2 = opool.tile([D, S], F16)  # head 2

            # ---- attention per head ----
            for h in range(HPC):
                qt, qo = qsrc[h]
                kt, ko = ksrc[h]
                # scoresT + exp -> pT tiles [128, S] fp16 (partition = Sk block)
                pts = []
                for sk in range(S // P):
                    pt = ptpool.tile([P, S], F16, name="pt", tag="pt")
                    pts.append(pt)
                    for g in range(2):
                        sps = psum.tile([P, 1024], F32, tag="sc", name="ps_s")
                        for j in range(2):
                            nc.tensor.matmul(
                                sps[:, ts(j, 512)],
                                lhsT=kt[ko : ko + D, ts(sk, P)],
                                rhs=qt[qo : qo + D, ds(g * 1024 + j * 512, 512)],
                                start=True,
                                stop=True,
                            )
                        nc.scalar.activation(
                            out=pt[:, ts(g, 1024)],
                            in_=sps,
                            func=mybir.ActivationFunctionType.Exp,
                            scale=float(D) ** -0.5,
                        )

                # AV^T: avs[nq] [65, 512] = sum_sk v'[sk]^T @ pT[sk][:, nq]
                avs = []
                for nq in range(NQ):
                    av = psum.tile([D + 1, 512], F32, tag="av", bufs=4, name="ps_av")
                    avs.append(av)
                for sk in range(S // P):
                    for nq in range(NQ):
                        nc.tensor.matmul(
                            avs[nq],
                            lhsT=vp[h][:, ds(sk * (D + 1), D + 1)],
                            rhs=pts[sk][:, ts(nq, 512)],
                            start=(sk == 0),
                            stop=(sk == S // P - 1),
                        )

                # normalize: out = av[0:64] / l, l = av[64]; 1/l is broadcast
                # across partitions via a DRAM round-trip (0-stride APs are
                # only legal on DRAM sources).
                rl = rlpool.tile([D + 1, S], F32, name="rl", tag="rl")
                for nq in range(NQ):
                    nc.vector.reciprocal(
                        out=rl[D : D + 1, ts(nq, 512)], in_=avs[nq][D : D + 1, :]
                    )
                scr = scrpool.tile([1, S], F32, name="scr", tag="scr")
                nc.sync.dma_start(out=scr, in_=rl[D : D + 1, :])
                rlb = rlpool.tile([D, S], F32, name="rlb", tag="rlb")
                nc.gpsimd.dma_start(out=rlb, in_=scr.to_broadcast([D, S]))
                if h == 0:
                    dsts = [st01[0:D, ts(nq, 512)] for nq in range(NQ)]
                elif h == 1:
                    dsts = [st01[D:P, ts(nq, 512)] for nq in range(NQ)]
                else:
                    dsts = [outT2[0:D, ts(nq, 512)] for nq in range(NQ)]
                for nq in range(NQ):
                    nc.vector.tensor_mul(
                        out=dsts[nq], in0=avs[nq][0:D, :], in1=rlb[:, ts(nq, 512)]
                    )

            # ---- output projection: yT [768, S] = wp^T @ out_headsT ----
            for mt in range(EMBED // P):
                for nq in range(NQ):
                    yps = psum.tile([P, 512], F32, tag="sc", name="ps_y")
                    nc.tensor.matmul(
                        yps,
                        lhsT=wpA[:, ts(mt, P)],
                        rhs=st01[:, ts(nq, 512)],
                        start=True,
                        stop=False,
                    )
                    nc.tensor.matmul(
                        yps,
                        lhsT=wpB[:, ts(mt, P)],
                        rhs=outT2[:, ts(nq, 512)],
                        start=False,
                        stop=True,
                    )
                    ysb = rlpool.tile([P, 512], F32, name="ysb", tag="ysb", bufs=3)
                    nc.vector.tensor_copy(out=ysb, in_=yps)
                    nc.sync.dma_start(out=yT[ts(mt, P), ts(nq, 512)], in_=ysb)
    return nc


_CACHE = threading.Lock(), {}


def _get_nc():
    lock, cache = _CACHE
    with lock:
        if "nc" not in cache:
            nc = bacc.Bacc("TRN2", target_bir_lowering=False, debug=False)
            _build_kernel(nc)
            nc.compile()
            cache["nc"] = nc
        return cache["nc"]


def _shard_inputs(x, w_qkv, b_qkv, w_proj):
    """Build the 8 per-core input maps (host-side sharding/layout)."""
    f16 = ml_dtypes.float16 if False else np.float16
    in_maps = []
    for c in range(NCORES):
        b = c // 4
        hg = c % 4
        h0 = HPC * hg
        # column order: q0 q1 k0 k1 q2 k2 (within the head group)
        cols = []
        for t in range(HPC):
            pass
        qcols = [np.arange(D * (h0 + i), D * (h0 + i + 1)) for i in range(HPC)]
        kcols = [EMBED + q for q in qcols]
        vcols = [2 * EMBED + q for q in qcols]
        order = np.concatenate(
            [qcols[0], qcols[1], kcols[0], kcols[1], qcols[2], kcols[2]]
        )
        vorder = np.concatenate(vcols)
        prows = np.concatenate(qcols)  # w_proj rows for these heads
        in_maps.append(
            {
                "xT": np.ascontiguousarray(x[b].T).astype(f16),
                "w_qk": np.ascontiguousarray(w_qkv[:, order]).astype(f16),
                "b_qk": np.ascontiguousarray(b_qkv[order]).astype(np.float32),
                "w_v": np.ascontiguousarray(w_qkv[:, vorder]).astype(f16),
                "w_p": np.ascontiguousarray(w_proj[prows, :]).astype(f16),
            }
        )
    return in_maps


def kernel(x, w_qkv, b_qkv, w_proj, b_proj, _results_hook=None):
    x = np.asarray(x, dtype=np.float32)
    w_qkv = np.asarray(w_qkv, dtype=np.float32)
    b_qkv = np.asarray(b_qkv, dtype=np.float32)
    w_proj = np.asarray(w_proj, dtype=np.float32)
    b_proj = np.asarray(b_proj, dtype=np.float32)

    nc = _get_nc()
    in_maps = _shard_inputs(x, w_qkv, b_qkv, w_proj)
    res = run_bass_kernel_spmd(nc, in_maps, core_ids=list(range(NCORES)))
    if _results_hook is not None:
        _results_hook(res)

    # unshard: sum the 4 head-group partials per batch, add bias terms
    b_v = b_qkv[2 * EMBED :]
    bias_row = b_v @ w_proj + b_proj  # [768]
    out = np.empty((B, S, EMBED), dtype=np.float32)
    for b in range(B):
        acc = np.zeros((EMBED, S), dtype=np.float32)
        for hg in range(4):
            acc += res.results[4 * b + hg]["yT"]
        out[b] = acc.T + bias_row
    return out


# revision 9
# speedup vs baseline: 1.2172x; 1.2172x over previous
"""Multi-head attention (B=2, S=2048, E=768, H=12, D=64) on 8 NeuronCores.

Sharding: core c -> batch b = c//4, head group hg = c%4 (3 heads each).
Each core computes the qkv projection for its 3 heads, attention, and a
partial output projection (rows of w_proj for its heads). Host sums the 4
partials per batch and adds the bias terms (tensor-parallel unshard).

Device dataflow (everything transposed so no on-chip transposes are needed):
  xT [768, 2048]  (host-pretransposed, fp16)
  qkT[t] = (w_qk_tile_t)^T @ xT   -> [128, 2048] tiles t=0..2, cols ordered
           [q0|q1], [k0|k1], [q2|k2]; k2 moved to partitions 0:64 by an
           SBUF->SBUF DMA so every head's q,k share a base partition.
  v'   = xT_tile^T @ w_v          -> per-head per-Sk-block [128, 128] blocks:
           cols 0:64 = v, cols 64:128 = 1.0, so the AV matmul also produces
           the softmax denominator replicated across 64 partitions.
  pT   = exp(kT_tile^T @ qT / 8)  (ScalarE, PSUM->SBUF fp16)
  avT  = v'^T @ pT                -> [128, 512] PSUM per (head, Sq-chunk);
           rows 64:128 = denominator l
  outT = avT[0:64] * approx(1/l)  (DVE, reciprocal_approx_fast ~51 ULP)
  yT  += w_proj_part^T @ outT     -> [768, 2048] fp32 partial, DMA'd out.

Emission order is tuned so that the first exp (ScalarE is the pacing
engine mid-kernel) starts as soon as the first two qkT tiles exist, and
each head's AV matmuls are emitted AFTER the next head's scores so the
Tile scheduler uses them to fill TensorE gaps while ScalarE consumes
score tiles.
"""

import threading

import numpy as np

import concourse.bass as bass
import concourse.tile as tile
from concourse import bacc, mybir
from concourse.bass import ts, ds
from concourse.bass_utils import run_bass_kernel_spmd

F32 = mybir.dt.float32
F16 = mybir.dt.float16

EMBED = 768
NH = 12
D = 64
B = 2
S = 2048
HPC = 3          # heads per core
NCORES = 8
P = 128
KC = EMBED // P  # 6 contraction chunks
NQ = S // 512    # 4 Sq chunks of 512
NSK = S // P     # 16 Sk blocks


def _build_kernel(nc):
    xT = nc.dram_tensor("xT", [EMBED, S], F16, kind="ExternalInput").ap()
    wqk = nc.dram_tensor("w_qk", [EMBED, 2 * HPC * D], F16, kind="ExternalInput").ap()
    bqk = nc.dram_tensor("b_qk", [2 * HPC * D], F32, kind="ExternalInput").ap()
    wv = nc.dram_tensor("w_v", [EMBED, HPC * D], F16, kind="ExternalInput").ap()
    wp = nc.dram_tensor("w_p", [HPC * D, EMBED], F16, kind="ExternalInput").ap()
    yT = nc.dram_tensor("yT", [EMBED, S], F32, kind="ExternalOutput").ap()

    with tile.TileContext(nc) as tc:
        with (
            tc.tile_pool(name="wpool", bufs=1) as wpool,
            tc.tile_pool(name="xpool", bufs=1) as xpool,
            tc.tile_pool(name="qkpool", bufs=1) as qkpool,
            tc.tile_pool(name="vpool", bufs=1) as vpool,
            tc.tile_pool(name="ptpool", bufs=24) as ptpool,
            tc.tile_pool(name="opool", bufs=1) as opool,
            tc.tile_pool(name="rlpool", bufs=4) as rlpool,
            tc.tile_pool(name="psum", bufs=2, space="PSUM") as psum,
        ):
            # ---- loads (xT interleaved with wqk so qk matmuls start early)
            wqk_t = []
            xT_t = []
            for k in range(KC):
                xT_k = xpool.tile([P, S], F16, name=f"xT{k}")
                nc.sync.dma_start(out=xT_k, in_=xT[ts(k, P), :])
                xT_t.append(xT_k)
                wqk_k = wpool.tile([P, 2 * HPC * D], F16, name=f"wqk{k}")
                nc.sync.dma_start(out=wqk_k, in_=wqk[ts(k, P), :])
                wqk_t.append(wqk_k)
            bq_t = []
            for t in range(HPC):
                bq_k = wpool.tile([P, 1], F32, name=f"bq{t}")
                nc.sync.dma_start(
                    out=bq_k, in_=bqk[ts(t, P)].rearrange("(p o) -> p o", o=1)
                )
                bq_t.append(bq_k)
            wv_t = []
            for k in range(KC):
                wv_k = wpool.tile([P, HPC * D], F16, name=f"wv{k}")
                nc.sync.dma_start(out=wv_k, in_=wv[ts(k, P), :])
                wv_t.append(wv_k)
            wpA = wpool.tile([P, EMBED], F16)
            nc.sync.dma_start(out=wpA, in_=wp[0:P, :])
            wpB = wpool.tile([D, EMBED], F16)
            nc.sync.dma_start(out=wpB, in_=wp[P : P + D, :])

            def qk_tile(t):
                qkT_i = qkpool.tile([P, S], F16, name=f"qkT{t}", tag=f"qkT{t}")
                for nq in range(NQ):
                    ps = psum.tile([P, 512], F32, tag="sc", name="ps_qk")
                    for k in range(KC):
                        nc.tensor.matmul(
                            ps,
                            lhsT=wqk_t[k][:, ts(t, P)],
                            rhs=xT_t[k][:, ts(nq, 512)],
                            start=(k == 0),
                            stop=(k == KC - 1),
                        )
                    nc.vector.tensor_scalar_add(
                        out=qkT_i[:, ts(nq, 512)], in0=ps, scalar1=bq_t[t]
                    )
                return qkT_i

            # ---- qkT tiles 0,1 (q and k for heads 0,1) ----
            qkT = [qk_tile(0), qk_tile(1)]

            def scores_head(qt, qo, kt, ko):
                """Emit scoresT+exp for one head; returns the 16 pT tiles."""
                pts = []
                for sk in range(NSK):
                    pt = ptpool.tile([P, S], F16, name="pt", tag="pt")
                    pts.append(pt)
                    for g in range(2):
                        sps = psum.tile([P, 1024], F32, tag="sc", name="ps_s")
                        for j in range(2):
                            nc.tensor.matmul(
                                sps[:, ts(j, 512)],
                                lhsT=kt[ko : ko + D, ts(sk, P)],
                                rhs=qt[qo : qo + D, ds(g * 1024 + j * 512, 512)],
                                start=True,
                                stop=True,
                            )
                        nc.scalar.activation(
                            out=pt[:, ts(g, 1024)],
                            in_=sps,
                            func=mybir.ActivationFunctionType.Exp,
                            scale=float(D) ** -0.5,
                        )
                return pts

            # scores for head 0 as early as possible (feeds ScalarE)
            pts_h = [scores_head(qkT[0], 0, qkT[1], 0)]

            # ---- rest of qkv: tile 2, k2 fixup, v' ----
            qkT.append(qk_tile(2))
            k2f = qkpool.tile([D, S], F16)
            nc.sync.dma_start(out=k2f, in_=qkT[2][D:P, :])

            vp = []
            for h in range(HPC):
                vp_h = vpool.tile([P, NSK * P], F16, name=f"vp{h}", tag=f"vp{h}")
                nc.vector.memset(
                    vp_h.rearrange("p (s c) -> p s c", c=P)[:, :, D:P], 1.0
                )
                vp.append(vp_h)
            for st in range(NSK):
                vps = psum.tile([P, HPC * D], F32, tag="sc", name="ps_v")
                for k in range(KC):
                    nc.tensor.matmul(
                        vps,
                        lhsT=xT_t[k][:, ts(st, P)],
                        rhs=wv_t[k],
                        start=(k == 0),
                        stop=(k == KC - 1),
                    )
                for h in range(HPC):
                    nc.vector.tensor_copy(
                        out=vp[h][:, ds(st * P, D)], in_=vps[:, ts(h, D)]
                    )

            qsrc = [(qkT[0], 0), (qkT[0], D), (qkT[2], 0)]
            ksrc = [(qkT[1], 0), (qkT[1], D), (k2f, 0)]

            st01 = opool.tile([P, S], F16)   # heads 0 (rows 0:64) and 1 (64:128)
            outT2 = opool.tile([D, S], F16)  # head 2

            def av_and_norm(h, pts):
                """AV^T accumulation + softmax normalization for head h."""
                avs = []
                for nq in range(NQ):
                    av = psum.tile([P, 512], F32, tag="av", bufs=4, name="ps_av")
                    avs.append(av)
                for sk in range(NSK):
                    for nq in range(NQ):
                        nc.tensor.matmul(
                            avs[nq],
                            lhsT=vp[h][:, ts(sk, P)],
                            rhs=pts[sk][:, ts(nq, 512)],
                            start=(sk == 0),
                            stop=(sk == NSK - 1),
                        )
                if h == 0:
                    dsts = [st01[0:D, ts(nq, 512)] for nq in range(NQ)]
                elif h == 1:
                    dsts = [st01[D:P, ts(nq, 512)] for nq in range(NQ)]
                else:
                    dsts = [outT2[0:D, ts(nq, 512)] for nq in range(NQ)]
                for nq in range(NQ):
                    # custom-DVE ops can't partition-shift: stage l at base 0
                    ll = rlpool.tile([D, 512], F32, name="ll", tag="ll")
                    nc.vector.tensor_copy(out=ll, in_=avs[nq][D:P, :])
                    rr = rlpool.tile([D, 512], F32, name="rr", tag="rr")
                    nc.vector.reciprocal_approx_fast(out=rr, in_=ll)
                    nc.vector.tensor_mul(
                        out=dsts[nq], in0=avs[nq][0:D, :], in1=rr
                    )

            # scores h+1 emitted BEFORE AV h: the scheduler then prefers to
            # keep ScalarE fed and uses AV matmuls to fill TensorE gaps.
            pts_h.append(scores_head(qkT[0], D, qkT[1], D))
            av_and_norm(0, pts_h[0])
            pts_h.append(scores_head(qkT[2], 0, k2f, 0))
            av_and_norm(1, pts_h[1])
            av_and_norm(2, pts_h[2])

            # ---- output projection: yT [768, S] = wp^T @ out_headsT ----
            for mt in range(EMBED // P):
                for nq in range(NQ):
                    yps = psum.tile([P, 512], F32, tag="sc", name="ps_y")
                    nc.tensor.matmul(
                        yps,
                        lhsT=wpA[:, ts(mt, P)],
                        rhs=st01[:, ts(nq, 512)],
                        start=True,
                        stop=False,
                    )
                    nc.tensor.matmul(
                        yps,
                        lhsT=wpB[:, ts(mt, P)],
                        rhs=outT2[:, ts(nq, 512)],
                        start=False,
                        stop=True,
                    )
                    ysb = rlpool.tile([P, 512], F32, name="ysb", tag="ysb", bufs=3)
                    nc.vector.tensor_copy(out=ysb, in_=yps)
                    nc.sync.dma_start(out=yT[ts(mt, P), ts(nq, 512)], in_=ysb)
    return nc


_CACHE = threading.Lock(), {}


def _get_nc():
    lock, cache = _CACHE
    with lock:
        if "nc" not in cache:
            nc = bacc.Bacc("TRN2", target_bir_lowering=False, debug=False)
            _build_kernel(nc)
            nc.compile()
            cache["nc"] = nc
        return cache["nc"]


def _shard_inputs(x, w_qkv, b_qkv, w_proj):
    """Build the 8 per-core input maps (host-side sharding/layout)."""
    in_maps = []
    for c in range(NCORES):
        b = c // 4
        hg = c % 4
        h0 = HPC * hg
        qcols = [np.arange(D * (h0 + i), D * (h0 + i + 1)) for i in range(HPC)]
        kcols = [EMBED + q for q in qcols]
        vcols = [2 * EMBED + q for q in qcols]
        # column order: q0 q1 | k0 k1 | q2 k2 (within the head group)
        order = np.concatenate(
            [qcols[0], qcols[1], kcols[0], kcols[1], qcols[2], kcols[2]]
        )
        vorder = np.concatenate(vcols)
        prows = np.concatenate(qcols)  # w_proj rows for these heads
        in_maps.append(
            {
                "xT": np.ascontiguousarray(x[b].T).astype(np.float16),
                "w_qk": np.ascontiguousarray(w_qkv[:, order]).astype(np.float16),
                "b_qk": np.ascontiguousarray(b_qkv[order]).astype(np.float32),
                "w_v": np.ascontiguousarray(w_qkv[:, vorder]).astype(np.float16),
                "w_p": np.ascontiguousarray(w_proj[prows, :]).astype(np.float16),
            }
        )
    return in_maps


def kernel(x, w_qkv, b_qkv, w_proj, b_proj, _results_hook=None):
    x = np.asarray(x, dtype=np.float32)
    w_qkv = np.asarray(w_qkv, dtype=np.float32)
    b_qkv = np.asarray(b_qkv, dtype=np.float32)
    w_proj = np.asarray(w_proj, dtype=np.float32)
    b_proj = np.asarray(b_proj, dtype=np.float32)

    nc = _get_nc()
    in_maps = _shard_inputs(x, w_qkv, b_qkv, w_proj)
    res = run_bass_kernel_spmd(nc, in_maps, core_ids=list(range(NCORES)))
    if _results_hook is not None:
        _results_hook(res)

    # unshard: sum the 4 head-group partials per batch, add bias terms
    b_v = b_qkv[2 * EMBED :]
    bias_row = b_v @ w_proj + b_proj  # [768]
    out = np.empty((B, S, EMBED), dtype=np.float32)
    for b in range(B):
        acc = np.zeros((EMBED, S), dtype=np.float32)
        for hg in range(4):
            acc += res.results[4 * b + hg]["yT"]
        out[b] = acc.T + bias_row
    return out
